# revision 21
# baseline (speedup 1.0000x reference)
"""FBP reconstructor on 8 TRN2 NeuronCores (Bass/Tile).

Pipeline (per core, angle-sharded):
  1. column sums of x with H-boundary masks  (PE matmul, colmask const)
  2. 9-tap 1D conv -> proj_sim^T [det, (b,a)] (PE matmul)
  3. Ram-Lak filter folded into one [det, det] matrix F = dftA.diag(r).dftB
     (host-precomputed, f64); a column-reversed twin yields the
     detector-reversed filtered signal
  4. per-octet angle tables via selector matmuls: channels 0-7 of each
     16-partition group hold T (2 mirror signs x 4 batches), channels
     8-15 hold reversed T
  5. phase-quantized interp tables TQ[q*256+k] = T[k] + (q+.5)/Q*D[k]
     (same build for direct and reversed rows) + one extra entry at
     index Q*256 for detector-edge-clipped pixels.  The go2 octet's
     table is built first so its gathers overlap the other build.
  6. backprojection gather via gpsimd ap_gather over the UPPER HALF of
     the image only: per index, channels 0-7 give the contribution at
     pixel (y,x) / mirror (y,255-x); channels 8-15 (reversed tables)
     give it at the point-reflected pixel (255-y,255-x) / (255-y,x).
     ap_gather wall time is ~27ns/idx regardless of d/channels, so
     halving the idx count via the reflection halves the runtime.
     Flip-symmetry (a <-> 179-a) shares idx lists; all idx tiles are
     preloaded to SBUF so the loop has no DMA dependencies.
  7. PE selector-matmul accumulation of 2 chunks x 16 channels into a
     [32, 2048] PSUM tile; evict once per go2 gather.

Host: shards angle pairs across cores, builds constants/indices,
merges the 8 partial outputs with the mirror/reflection flips, scales.
"""

import sys

if "/opt/trn_rl_repo" not in sys.path:
    sys.path.insert(0, "/opt/trn_rl_repo")

import numpy as np

IMG = 256
DET = 256
NA = 180
NB = 4
NFREQ = DET // 2 + 1  # 129
Q = 64
NE = Q * DET + 1  # 16385 table elems per channel (last = clip entry)
MSPECIAL = Q * DET  # index of the clip entry
CHUNK = 2048
NCH = IMG * IMG // (2 * CHUNK)  # 16 chunks cover the upper half
NPAIR = NA // 2  # 90 flip-sym base pairs
NCORES = 8
BA = NB * NA  # 720
MT = 120  # ba M-tile size (6 tiles)
BLOBW = 4096  # packed const blob columns

_cache = {}


def _host_constants():
    """Input-independent constants."""
    # colmask[y, ki]: column-sum masks for the three H-offsets of the
    # 3x3 conv after summing over H (SAME padding).
    colmask = np.ones((IMG, 3), dtype=np.float32)
    colmask[IMG - 1, 0] = 0.0
    colmask[0, 2] = 0.0

    # Ram-Lak filter as a single [det, det] matrix (f64 compose).
    n = np.arange(DET)[:, None]
    k = np.arange(NFREQ)[None, :]
    ph = 2.0 * np.pi * n * k / DET
    dAc = np.cos(ph)
    dAs = np.sin(ph)
    u = np.full(NFREQ, 2.0 / DET)
    u[0] = 1.0 / DET
    u[NFREQ - 1] = 1.0 / DET
    phB = 2.0 * np.pi * np.arange(NFREQ)[:, None] * np.arange(DET)[None, :] / DET
    dBc = u[:, None] * np.cos(phB)
    dBs = u[:, None] * np.sin(phB)

    # accumulation selectors [3][128, 32]: v0/v1 route the two go
    # gathers to chunk slots 0/1; v2 routes go2's even groups to slot 0
    # and odd groups to slot 1 in a single matmul (disjoint columns).
    sela = np.zeros((3, 128, 32), dtype=np.float32)
    for g in range(8):
        for c in range(16):
            sela[0, 16 * g + c, c] = 1.0
            sela[1, 16 * g + c, 16 + c] = 1.0
            sela[2, 16 * g + c, (0 if g % 2 == 0 else 16) + c] = 1.0

    # pixel interp indices per base angle, UPPER half: idx = q*256 + i0,
    # with detector-edge clips (i0 == 255) remapped to the special entry.
    lin = np.linspace(-1.0, 1.0, IMG).astype(np.float32)
    yy, xx = np.meshgrid(lin, lin, indexing="ij")
    ang = np.linspace(0.0, np.pi, NA).astype(np.float32)
    npx = IMG * IMG // 2
    idxq = np.empty((NPAIR, npx), dtype=np.int16)
    for i in range(NPAIR):
        c = np.float32(np.cos(ang[i]))
        s = np.float32(np.sin(ang[i]))
        t = (c * xx[:IMG // 2] + s * yy[:IMG // 2]).astype(np.float32)
        ix = np.clip((t + np.float32(1.0)) * np.float32(0.5) * np.float32(DET - 1),
                     0.0, DET - 1).astype(np.float32)
        i0 = np.floor(ix).astype(np.int32)
        w = (ix - i0).astype(np.float32)
        qb = np.minimum((w * Q).astype(np.int32), Q - 1)
        mi = qb * DET + i0
        mi[i0 == DET - 1] = MSPECIAL
        idxq[i] = mi.astype(np.int16).reshape(-1)

    ramlak_holder = {"dAc": dAc, "dAs": dAs, "dBc": dBc, "dBs": dBs}
    return colmask, ramlak_holder, sela, idxq


def _wrap16(flat_idx):
    """[2048] pixel idx -> [16, 128] wrapped layout (j at partition j%16, col j//16)."""
    return flat_idx.reshape(128, 16).T  # j = u*16 + l  ->  [l, u]


def _sel_pair(slots):
    """Selector [BA, 128] x2 (direct, reversed rows) for (group, pair) slots."""
    sv = np.zeros((BA, 128), dtype=np.float32)
    sw = np.zeros((BA, 128), dtype=np.float32)
    for g, pi in slots:
        if pi < 0:
            continue
        for s in range(2):
            a = pi if s == 0 else NA - 1 - pi
            for b in range(NB):
                sv[b * NA + a, 16 * g + 4 * s + b] = 1.0
                sw[b * NA + a, 16 * g + 8 + 4 * s + b] = 1.0
    return sv, sw


def _per_core_inputs(idxq):
    """Per-core SEL matrices and wrapped idx arrays (SBUF-preload layout)."""
    out = []
    for r in range(NCORES):
        pairs = list(range(r, NPAIR, NCORES))  # 11 or 12
        p1 = pairs[:8]
        p2 = pairs[8:]
        while len(p2) < 4:
            p2.append(-1)  # dummy slot

        sel1v, sel1w = _sel_pair(list(enumerate(p1)))
        slots2 = []
        for p, pi in enumerate(p2):
            for h in range(2):
                slots2.append((2 * p + h, pi))
        sel2v, sel2w = _sel_pair(slots2)

        idx1 = np.zeros((NCH, 128, 128), dtype=np.int16)
        for ci in range(NCH):
            for g, pi in enumerate(p1):
                w = _wrap16(idxq[pi, ci * CHUNK:(ci + 1) * CHUNK])
                idx1[ci, 16 * g:16 * g + 16, :] = w
        idx2 = np.zeros((NCH // 2, 128, 128), dtype=np.int16)
        for cc in range(NCH // 2):
            for p, pi in enumerate(p2):
                if pi < 0:
                    continue
                for h in range(2):
                    ci = 2 * cc + h
                    w = _wrap16(idxq[pi, ci * CHUNK:(ci + 1) * CHUNK])
                    idx2[cc, 16 * (2 * p + h):16 * (2 * p + h) + 16, :] = w
        idx1 = np.ascontiguousarray(idx1.transpose(1, 0, 2))
        idx2 = np.ascontiguousarray(idx2.transpose(1, 0, 2))
        out.append((sel1v, sel1w, sel2v, sel2w, idx1, idx2))
    return out


def _build_blobs(ramlak, holder, percore):
    """Per-core packed const blob [128, 4096]: F halves, F_rev halves, sels."""
    r = np.asarray(ramlak, dtype=np.float64)
    F = (holder["dAc"] @ (r[:, None] * holder["dBc"])
         + holder["dAs"] @ (r[:, None] * holder["dBs"])).astype(np.float32)
    Fr = np.ascontiguousarray(F[:, ::-1])
    blobs = []
    for core in percore:
        sel1v, sel1w, sel2v, sel2w = core[:4]
        blob = np.zeros((128, BLOBW), dtype=np.float32)
        blob[:, 0:256] = F[0:128]
        blob[:, 256:512] = F[128:256]
        blob[:, 512:768] = Fr[0:128]
        blob[:, 768:1024] = Fr[128:256]
        for si, sel in enumerate((sel1v, sel1w, sel2v, sel2w)):
            for t in range(6):
                c0 = 1024 + 768 * si + 128 * t
                blob[0:MT, c0:c0 + 128] = sel[MT * t:MT * (t + 1), :]
        blobs.append(blob)
    return blobs


def _build_nc():
    import concourse.mybir as mybir
    import concourse.tile as tile
    from concourse import bacc

    f32 = mybir.dt.float32
    i16 = mybir.dt.int16
    mult = mybir.AluOpType.mult
    add = mybir.AluOpType.add

    nc = bacc.Bacc(None, target_bir_lowering=False, debug=False)

    x_d = nc.dram_tensor("x3", [NB, IMG, IMG], f32, kind="ExternalInput")
    w9_d = nc.dram_tensor("w9", [3, 3, NA], f32, kind="ExternalInput")
    cm_d = nc.dram_tensor("colmask", [IMG, 3], f32, kind="ExternalInput")
    blob_d = nc.dram_tensor("blob", [128, BLOBW], f32, kind="ExternalInput")
    sela_d = nc.dram_tensor("sela", [128, 3, 32], f32, kind="ExternalInput")
    idx1_d = nc.dram_tensor("idx1", [128, NCH, 128], i16, kind="ExternalInput")
    idx2_d = nc.dram_tensor("idx2", [128, NCH // 2, 128], i16,
                            kind="ExternalInput")
    out_d = nc.dram_tensor("partial", [NCH // 2, 32, CHUNK], f32,
                           kind="ExternalOutput")

    with tile.TileContext(nc) as tc:
        with tc.tile_pool(name="persist", bufs=1) as pp:
            TQ1_t = pp.tile([128, NE], f32)  # go2 octet (built first)
            TQ0_t = pp.tile([128, NE], f32)
            idx1_t = pp.tile([128, NCH, 128], i16)
            idx2_t = pp.tile([128, NCH // 2, 128], i16)
            sela_t = pp.tile([128, 3, 32], f32)
            # octet T/D live here so the gather-loop pools (which reuse
            # the setup pool's bytes) don't overlap the STT sources --
            # that WAR dependency would delay the first gather.
            T_oct = pp.tile([128, 2, DET], f32)
            D_oct = pp.tile([128, 2, DET], f32)
            # chunk 0/1 gather outputs: persistent tiles, so the first
            # two gathers can be issued between the two table builds
            # (their semaphore wait then covers only TQ0's build ops).
            go0_t = pp.tile([128, CHUNK], f32)
            go1_t = pp.tile([128, CHUNK], f32)
            # idx preload first: on the sync queue ahead of any
            # pool-release semaphores, so the first gather's inputs are
            # resident long before the tables finish building.
            nc.sync.dma_start(out=idx2_t[:], in_=idx2_d[:, :, :])
            nc.sync.dma_start(out=idx1_t[:], in_=idx1_d[:, :, :])
            nc.sync.dma_start(out=sela_t[:], in_=sela_d[:, :, :])

            with tc.tile_pool(name="setup", bufs=1) as sp:
                w9_t = sp.tile([3, 3, NA], f32)
                cm_t = sp.tile([128, 2, 3], f32)
                blob_t = sp.tile([128, BLOBW], f32)
                nc.sync.dma_start(out=cm_t[:, 0], in_=cm_d[0:128, :])
                nc.sync.dma_start(out=cm_t[:, 1], in_=cm_d[128:256, :])
                nc.sync.dma_start(out=w9_t[:], in_=w9_d[:, :, :])
                nc.sync.dma_start(out=blob_t[:], in_=blob_d[:, :])

                def F_ap(rev, kt):
                    c0 = 512 * rev + 256 * kt
                    return blob_t[:, c0:c0 + 256]

                def sel_ap(si, t):
                    c0 = 1024 + 768 * si + 128 * t
                    return blob_t[0:MT, c0:c0 + 128]

                pT_sb = sp.tile([128, 2, BA], f32)  # proj_sim^T [det, (b,a)]

                with (
                    tc.tile_pool(name="xload", bufs=2) as xp,
                    tc.tile_pool(name="ps_a", bufs=2, space="PSUM") as psa,
                ):
                    for b in range(NB):
                        xt = xp.tile([128, 2, IMG], f32)
                        nc.sync.dma_start(out=xt[:, 0], in_=x_d[b, 0:128, :])
                        nc.sync.dma_start(out=xt[:, 1], in_=x_d[b, 128:256, :])
                        tk_ps = psa.tile([3, IMG], f32)
                        nc.tensor.matmul(tk_ps[:], cm_t[:, 0], xt[:, 0],
                                         start=True, stop=False)
                        nc.tensor.matmul(tk_ps[:], cm_t[:, 1], xt[:, 1],
                                         start=False, stop=True)
                        # zero-padded column sums: tk3[_, 1+v] = Tk[v]
                        tk3_t = xp.tile([3, IMG + 2], f32)
                        nc.vector.memset(tk3_t[:], 0.0)
                        nc.scalar.copy(out=tk3_t[:, 1:IMG + 1], in_=tk_ps[:])
                        for jt in range(2):
                            pc_ps = psa.tile([128, NA], f32)
                            for kj in range(3):
                                nc.tensor.matmul(
                                    pc_ps[:],
                                    tk3_t[:, jt * 128 + kj:jt * 128 + kj + 128],
                                    w9_t[:, kj],
                                    start=(kj == 0), stop=(kj == 2))
                            nc.scalar.copy(
                                out=pT_sb[:, jt, b * NA:(b + 1) * NA],
                                in_=pc_ps[:])

                # ---- filtered projections, direct + reversed, via F matmul
                filt_sb = sp.tile([MT, 2, 6, DET], f32)  # [m, rev, mt, det]
                with tc.tile_pool(name="ps_c", bufs=2, space="PSUM") as psc:
                    for rev in range(2):
                        for mt in range(6):
                            ms = slice(mt * MT, (mt + 1) * MT)
                            f_ps = psc.tile([MT, DET], f32)
                            for kt in range(2):
                                nc.tensor.matmul(f_ps[:], pT_sb[:, kt, ms],
                                                 F_ap(rev, kt),
                                                 start=(kt == 0), stop=(kt == 1))
                            nc.scalar.copy(out=filt_sb[:, rev, mt], in_=f_ps[:])

                # ---- per-octet tables and TQ build; go octet (o=0) first
                # (the gather loop leads with go gathers, so TQ0's build
                # is the critical path and TQ1's overlaps the gathers)
                with tc.tile_pool(name="ps_d", bufs=2, space="PSUM") as psd:
                    for o, TQ_t in ((0, TQ0_t), (1, TQ1_t)):
                        si_v = 2 * o
                        t_ps = psd.tile([128, DET], f32)
                        for kt in range(6):
                            nc.tensor.matmul(t_ps[:], sel_ap(si_v, kt),
                                             filt_sb[:, 0, kt, :],
                                             start=(kt == 0), stop=False)
                        for kt in range(6):
                            nc.tensor.matmul(t_ps[:], sel_ap(si_v + 1, kt),
                                             filt_sb[:, 1, kt, :],
                                             start=False, stop=(kt == 5))
                        T_o = T_oct[:, o]
                        D_o = D_oct[:, o]
                        nc.scalar.copy(out=T_o[:], in_=t_ps[:])
                        nc.vector.tensor_sub(D_o[:, 0:DET - 1], T_o[:, 1:DET],
                                             T_o[:, 0:DET - 1])
                        nc.vector.memset(D_o[:, DET - 1:DET], 0.0)
                        # clip entry: table[Q*256] = T[255] (rev rows: T[0])
                        nc.scalar.copy(out=TQ_t[:, MSPECIAL:MSPECIAL + 1],
                                       in_=T_o[:, DET - 1:DET])
                        for qq in range(Q):
                            nc.vector.scalar_tensor_tensor(
                                TQ_t[:, qq * DET:(qq + 1) * DET],
                                D_o[:], float((qq + 0.5) / Q), T_o[:],
                                mult, add)
                        if o == 0:
                            # chunk 0/1 gathers, ahead of TQ1's build in
                            # program order: they start as soon as TQ0
                            # is ready instead of after both builds.
                            nc.gpsimd.ap_gather(
                                go0_t[:], TQ0_t[:], idx1_t[:, 0, :],
                                channels=128, num_elems=NE,
                                d=1, num_idxs=CHUNK)
                            nc.gpsimd.ap_gather(
                                go1_t[:], TQ0_t[:], idx1_t[:, 1, :],
                                channels=128, num_elems=NE,
                                d=1, num_idxs=CHUNK)

            # ---- gather + accumulate (no DMA deps inside the loop)
            with (
                tc.tile_pool(name="gout", bufs=3) as gop,
                tc.tile_pool(name="gout2", bufs=2) as gop2,
                tc.tile_pool(name="evb", bufs=1) as evp,
                tc.tile_pool(name="ps_acc", bufs=2, space="PSUM") as psacc,
            ):
                def gather_go(ci):
                    go = gop.tile([128, CHUNK], f32, tag="go", name="go")
                    nc.gpsimd.ap_gather(go[:], TQ0_t[:], idx1_t[:, ci, :],
                                        channels=128, num_elems=NE,
                                        d=1, num_idxs=CHUNK)
                    return go

                def gather_go2(cc):
                    go2 = gop2.tile([128, CHUNK], f32, tag="go2", name="go2")
                    nc.gpsimd.ap_gather(go2[:], TQ1_t[:], idx2_t[:, cc, :],
                                        channels=128, num_elems=NE,
                                        d=1, num_idxs=CHUNK)
                    return go2

                for cc in range(NCH // 2):
                    last = cc == NCH // 2 - 1
                    if last:
                        # go2 first so the final chunk's accumulation
                        # overlaps its own go gathers (shorter tail)
                        go2 = gather_go2(cc)
                        gos = [gather_go(2 * cc), gather_go(2 * cc + 1)]
                    else:
                        if cc == 0:
                            gos = [go0_t, go1_t]  # issued during setup
                        else:
                            gos = [gather_go(2 * cc), gather_go(2 * cc + 1)]
                        go2 = gather_go2(cc)
                    acc = psacc.tile([32, CHUNK], f32)
                    for j in range(4):
                        js = slice(512 * j, 512 * (j + 1))
                        order = ((2, go2), (0, gos[0]), (1, gos[1])) if last \
                            else ((0, gos[0]), (1, gos[1]), (2, go2))
                        for n, (v, src) in enumerate(order):
                            nc.tensor.matmul(acc[:, js], sela_t[:, v, :],
                                             src[:, js],
                                             start=(n == 0), stop=(n == 2))
                    ev = evp.tile([32, CHUNK], f32, tag="ev")
                    nc.scalar.copy(out=ev[:], in_=acc[:])
                    nc.sync.dma_start(out=out_d[cc], in_=ev[:])
    nc.compile()
    return nc


def _get_compiled():
    if "nc" not in _cache:
        colmask, holder, sela, idxq = _host_constants()
        _cache["consts"] = (colmask, holder, sela)
        _cache["percore"] = _per_core_inputs(idxq)
        _cache["nc"] = _build_nc()
    return _cache["nc"], _cache["consts"], _cache["percore"]


def _in_maps(x, conv_w, ramlak):
    nc, consts, percore = _get_compiled()
    colmask, holder, sela = consts
    blobs = _build_blobs(ramlak, holder, percore)

    x3 = np.ascontiguousarray(np.asarray(x, dtype=np.float32).reshape(NB, IMG, IMG))
    # w9[ki, kj, a] = conv_w[a, 0, ki, kj]; device tile partition axis = ki
    w9 = np.ascontiguousarray(
        np.asarray(conv_w, dtype=np.float32).reshape(NA, 3, 3).transpose(1, 2, 0))

    common = {
        "x3": x3, "w9": w9, "colmask": colmask,
        "sela": np.ascontiguousarray(sela.transpose(1, 0, 2)),
    }
    in_maps = []
    for r_ in range(NCORES):
        idx1, idx2 = percore[r_][4], percore[r_][5]
        m = dict(common)
        m.update({"blob": blobs[r_], "idx1": idx1, "idx2": idx2})
        in_maps.append(m)
    return nc, in_maps


def kernel(x, conv_w, ramlak):
    from concourse.bass_utils import run_bass_kernel_spmd

    nc, in_maps = _in_maps(x, conv_w, ramlak)
    res = run_bass_kernel_spmd(nc, in_maps, list(range(NCORES)))

    npx = IMG * IMG // 2
    total = np.zeros((16, npx), dtype=np.float32)
    for r_ in range(NCORES):
        part = res.results[r_]["partial"]  # [8 cc, 32 (2 slot x 16 ch), 2048]
        total += part.reshape(NCH // 2, 2, 16, CHUNK).transpose(
            2, 0, 1, 3).reshape(16, npx)

    h = IMG // 2
    img = np.zeros((NB, IMG, IMG), dtype=np.float32)
    for b in range(NB):
        up0 = total[b].reshape(h, IMG)        # angle a at (y, x)
        up1 = total[4 + b].reshape(h, IMG)    # 179-a at (y, 255-x)
        rf0 = total[8 + b].reshape(h, IMG)    # a at (255-y, 255-x)
        rf1 = total[12 + b].reshape(h, IMG)   # 179-a at (255-y, x)
        img[b][:h] = up0 + up1[:, ::-1]
        img[b][h:] = rf0[::-1, ::-1] + rf1[::-1, :]
    out = img * np.float32(np.pi / NA)
    return np.ascontiguousarray(out.reshape(NB, 1, IMG, IMG)).astype(np.float32)
